# revision 2
# baseline (speedup 1.0000x reference)
"""3x3 median blur on Trainium2, data-parallel across 8 NeuronCores.

Input:  image (16, 3, 512, 512) float32
Output: median-blur(3x3, zero-padded) same shape.

Strategy:
- Shard batch across 8 cores: core c handles images [2c, 2c+2) -> 6 channel
  planes of 512x512 each.
- Host pads each plane to 514x514 with zeros (matches zero-pad semantics and
  removes all border special-casing on device).
- Device kernel (per core): 3 passes; each pass handles 2 planes. 128
  partitions x (64 row-chunks of 8 output rows per plane). Each partition
  holds a 10-row x 514-col slab (8 output rows + 1 halo row each side).
- Exact fp32 median-of-9 via separable sorting network on the vector engine:
    vertical: sort each 3-tall column into (lo, mid, hi) using shared
      adjacent-row min/max pairs;
    horizontal: median9 = med3(max3(lo), med3(mid), min3(hi)) with shared
      even/odd column pairs.
  ~15 tensor_tensor min/max ops per output pixel, all fp32-exact.
"""

import sys

if "/opt/trn_rl_repo" not in sys.path:
    sys.path.insert(0, "/opt/trn_rl_repo")

import numpy as np

import concourse.bass as bass
import concourse.tile as tile
from concourse import bacc, mybir
from concourse.bass_utils import run_bass_kernel_spmd

F32 = mybir.dt.float32
MAX = mybir.AluOpType.max
MIN = mybir.AluOpType.min

N_CORES = 8
B, C, H, W = 16, 3, 512, 512
PLANES = (B * C) // N_CORES  # 6 planes per core
PH, PW = H + 2, W + 2  # 514, 514
PLANE = PH * PW  # padded plane elems
OPLANE = H * W  # output plane elems

N_PASSES = PLANES // 2  # 2 planes per pass
CHUNK = 8  # output rows per partition per pass
SLAB = CHUNK + 2  # input rows per partition slab
SUB = 4  # output rows per sub-pass (2 sub-passes per pass)

_CACHED = {}


def _ap(apref, off, dims):
    """View into a tile/dram AP with explicit [step, num] free dims."""
    part = list(apref.ap[0])
    return bass.AP(apref.tensor, apref.offset + off, [part] + [list(d) for d in dims])


def _dram(handle, off, dims):
    return bass.AP(handle, off, [list(d) for d in dims])


def _build():
    nc = bacc.Bacc(
        "TRN2", target_bir_lowering=False, debug=False, num_devices=N_CORES
    )
    xin = nc.dram_tensor("xpad", [PLANES, PH, PW], F32, kind="ExternalInput")
    yout = nc.dram_tensor("y", [PLANES, H, W], F32, kind="ExternalOutput")

    with tile.TileContext(nc) as tc:
        _body(tc, nc, xin, yout)

    nc.compile()
    return nc


def _body(tc, nc, xin, yout):
    from contextlib import ExitStack

    ctx = ExitStack()
    with ctx:
        xpool = ctx.enter_context(tc.tile_pool(name="xpool", bufs=2))
        vpool = ctx.enter_context(tc.tile_pool(name="vpool", bufs=1))
        lmh = ctx.enter_context(tc.tile_pool(name="lmh", bufs=1))
        hpool = ctx.enter_context(tc.tile_pool(name="hpool", bufs=1))
        abc = ctx.enter_context(tc.tile_pool(name="abc", bufs=1))
        fin = ctx.enter_context(tc.tile_pool(name="fin", bufs=1))
        opool = ctx.enter_context(tc.tile_pool(name="opool", bufs=2))

        tt = nc.vector.tensor_tensor

        for t in range(N_PASSES):
            X = xpool.tile([128, SLAB * PW], F32, name="X")
            # input DMA: partition p (0..63) <- plane 2t rows 8p..8p+10;
            # partitions 64..127 <- plane 2t+1.
            for h in range(2):
                nc.sync.dma_start(
                    X[64 * h : 64 * h + 64, :],
                    _dram(
                        xin,
                        (2 * t + h) * PLANE,
                        [[CHUNK * PW, 64], [1, SLAB * PW]],
                    ),
                )

            for sp in range(2):
                b = sp * SUB  # slab row base for this sub-pass
                # X row views (within the slab): step is rows of PW elems
                def xv(r0, nrows, rstep=2):
                    return _ap(X, (b + r0) * PW, [[rstep * PW, nrows], [1, PW]])

                # ---- vertical stage: column sort3 -> lo, mid, hi ----
                # pairs at slab rows (b+1,b+2) and (b+3,b+4)
                pmin = vpool.tile([128, 2 * PW], F32, name="pmin")
                pmax = vpool.tile([128, 2 * PW], F32, name="pmax")
                pv = [[PW, 2], [1, PW]]
                tt(_ap(pmin, 0, pv), xv(1, 2), xv(2, 2), MIN)
                tt(_ap(pmax, 0, pv), xv(1, 2), xv(2, 2), MAX)

                lo = lmh.tile([128, SUB * PW], F32, name="lo")
                mid = lmh.tile([128, SUB * PW], F32, name="mid")
                hi = lmh.tile([128, SUB * PW], F32, name="hi")
                uo = vpool.tile([128, 2 * PW], F32, name="uo")
                ue = vpool.tile([128, 2 * PW], F32, name="ue")

                # tile rows r=0..3 <-> output slab rows s=b+1+r
                def lv(tl, r0):  # rows {r0, r0+2} of a SUB-row tile
                    return _ap(tl, r0 * PW, [[2 * PW, 2], [1, PW]])

                pm = _ap(pmin, 0, pv)
                pM = _ap(pmax, 0, pv)
                # odd outputs (tile rows 0,2): third element = X[s-1] = xv(0,2)
                # even outputs (tile rows 1,3): third = X[s+1] = xv(3,2)
                tt(lv(lo, 0), pm, xv(0, 2), MIN)
                tt(lv(lo, 1), pm, xv(3, 2), MIN)
                tt(lv(hi, 0), pM, xv(0, 2), MAX)
                tt(lv(hi, 1), pM, xv(3, 2), MAX)
                tt(_ap(uo, 0, pv), pM, xv(0, 2), MIN)
                tt(_ap(ue, 0, pv), pM, xv(3, 2), MIN)
                tt(lv(mid, 0), pm, _ap(uo, 0, pv), MAX)
                tt(lv(mid, 1), pm, _ap(ue, 0, pv), MAX)

                # ---- horizontal stage ----
                # even/odd column pairs over 514 cols -> 257 pairs
                NP = PW // 2  # 257
                def cview(tl, c0, ncols, cstep=2):
                    return _ap(tl, c0, [[PW, SUB], [cstep, ncols]])

                def pview(tl, k0, nk):
                    return _ap(tl, k0, [[NP, SUB], [1, nk]])

                mlo = hpool.tile([128, SUB * NP], F32, name="mlo")
                mhi = hpool.tile([128, SUB * NP], F32, name="mhi")
                pmn = hpool.tile([128, SUB * NP], F32, name="pmn")
                pmx = hpool.tile([128, SUB * NP], F32, name="pmx")

                tt(pview(mlo, 0, NP), cview(lo, 0, NP), cview(lo, 1, NP), MAX)
                tt(pview(mhi, 0, NP), cview(hi, 0, NP), cview(hi, 1, NP), MIN)
                tt(pview(pmn, 0, NP), cview(mid, 0, NP), cview(mid, 1, NP), MIN)
                tt(pview(pmx, 0, NP), cview(mid, 0, NP), cview(mid, 1, NP), MAX)

                # output-column views of W-wide tiles (row stride W)
                def ov(tl, c0, ncols, cstep=2):
                    return _ap(tl, c0, [[W, SUB], [cstep, ncols]])

                A = abc.tile([128, SUB * W], F32, name="A")
                Bt = abc.tile([128, SUB * W], F32, name="Bt")
                Ct = abc.tile([128, SUB * W], F32, name="Ct")
                ube = hpool.tile([128, SUB * (W // 2)], F32, name="ube")
                ubo = hpool.tile([128, SUB * (W // 2)], F32, name="ubo")
                NH = W // 2  # 256

                def uv(tl):
                    return _ap(tl, 0, [[NH, SUB], [1, NH]])

                # A = sliding max3 of lo; out col j (0-based output coords)
                # j even: max(mlo[j/2], lo[j+2]); j odd: max(mlo[(j+1)/2], lo[j])
                tt(ov(A, 0, NH), pview(mlo, 0, NH), cview(lo, 2, NH), MAX)
                tt(ov(A, 1, NH), pview(mlo, 1, NH), cview(lo, 1, NH), MAX)
                # C = sliding min3 of hi
                tt(ov(Ct, 0, NH), pview(mhi, 0, NH), cview(hi, 2, NH), MIN)
                tt(ov(Ct, 1, NH), pview(mhi, 1, NH), cview(hi, 1, NH), MIN)
                # B = sliding med3 of mid: med3(a, pair) = max(pmn, min(a, pmx))
                tt(uv(ube), cview(mid, 2, NH), pview(pmx, 0, NH), MIN)
                tt(ov(Bt, 0, NH), pview(pmn, 0, NH), uv(ube), MAX)
                tt(uv(ubo), cview(mid, 1, NH), pview(pmx, 1, NH), MIN)
                tt(ov(Bt, 1, NH), pview(pmn, 1, NH), uv(ubo), MAX)

                # ---- final med3(A, B, C) ----
                flat = [[1, SUB * W]]
                mn = fin.tile([128, SUB * W], F32, name="mn")
                mx = fin.tile([128, SUB * W], F32, name="mx")
                t2 = fin.tile([128, SUB * W], F32, name="t2")
                res = opool.tile([128, SUB * W], F32, name="res")
                tt(_ap(mn, 0, flat), _ap(A, 0, flat), _ap(Bt, 0, flat), MIN)
                tt(_ap(mx, 0, flat), _ap(A, 0, flat), _ap(Bt, 0, flat), MAX)
                tt(_ap(t2, 0, flat), _ap(mx, 0, flat), _ap(Ct, 0, flat), MIN)
                tt(_ap(res, 0, flat), _ap(mn, 0, flat), _ap(t2, 0, flat), MAX)

                # output DMA: partition p -> plane, rows 8*(p%64)+4*sp..+4
                for h in range(2):
                    nc.sync.dma_start(
                        _dram(
                            yout,
                            (2 * t + h) * OPLANE + sp * SUB * W,
                            [[CHUNK * W, 64], [W, SUB], [1, W]],
                        ),
                        res[64 * h : 64 * h + 64, :],
                    )


def _get_nc():
    if "nc" not in _CACHED:
        _CACHED["nc"] = _build()
    return _CACHED["nc"]


def kernel(image: np.ndarray, _trace: bool = False):
    assert image.shape == (B, C, H, W) and image.dtype == np.float32
    nc = _get_nc()

    in_maps = []
    per_core = B // N_CORES
    for c in range(N_CORES):
        shard = image[c * per_core : (c + 1) * per_core].reshape(PLANES, H, W)
        padded = np.zeros((PLANES, PH, PW), dtype=np.float32)
        padded[:, 1:-1, 1:-1] = shard
        in_maps.append({"xpad": padded})

    res = run_bass_kernel_spmd(
        nc, in_maps, list(range(N_CORES)), trace=_trace
    )
    _CACHED["last_exec_ns"] = res.exec_time_ns

    out = np.empty((B, C, H, W), dtype=np.float32)
    for c in range(N_CORES):
        out[c * per_core : (c + 1) * per_core] = res.results[c]["y"].reshape(
            per_core, C, H, W
        )
    return out
